# revision 93
# baseline (speedup 1.0000x reference)
"""Trainium2 Bass kernel for nn_AnyTSRpp (sparse_attention).

Strategy: pure data-parallel over the HR pixel grid (65536 px/batch),
8192 px/batch/core on 8 NeuronCores. Host computes per-pixel corner
indices/scalars; device gathers feat rows directly (per-corner indirect
DMA, pixel-major), applies the RBF weight per-partition pre-transpose,
PE transposes to channel-major, runs all matmuls/relu/softmax/gelu, and
a tiny AllReduce for the global attention logits (contraction over all
pixels). off_t = attn_t @ v_t is folded as (W00_off_t @ attn_t) @ v_t
so the attention output is never materialized.

Self-contained: hardcodes all shapes. kernel(**inputs) -> np.ndarray.
"""

import functools
import numpy as np
import ml_dtypes

BF16 = ml_dtypes.bfloat16


def _setup_jax_cache():
    """Persistent XLA compilation cache: repeated/every-process calls skip
    the neuronx-cc recompile of the identical kernel graph."""
    import jax
    try:
        jax.config.update('jax_compilation_cache_dir', '/root/.cache/jax_pcache')
        jax.config.update('jax_persistent_cache_min_compile_time_secs', 0.0)
        jax.config.update('jax_persistent_cache_min_entry_size_bytes', 0)
    except Exception:
        pass


_setup_jax_cache()

NCORES = 8
B = 2
C = 64
HLR = WLR = 64
HQ = WQ = 256
NPB = HQ * WQ            # 65536 pixels per batch
NLOC = NPB // NCORES     # 8192 pixels per batch per core
NROW = HLR * WLR         # 4096 feat rows (y-major)
CHUNK = 512              # matmul moving-N chunk
NCHUNK = NLOC // CHUNK   # 16
PCH = 512                # MLP pixel super-chunk
EPS = np.float32(1e-6)

# bf16 weight blob layout (flat element offsets)
WOFF_WQ = 0                        # [2, 64]  Wq.T/QK
WOFF_BQ = WOFF_WQ + 2 * 64         # [1, 64]
WOFF_WK = WOFF_BQ + 64             # [64, 64] Wk.T
WOFF_BK = WOFF_WK + 64 * 64        # [1, 64]
WOFF_WV = WOFF_BK + 64             # [64, 64]
WOFF_W00O = WOFF_WV + 64 * 64      # [4, 64, 256]
WOFF_W00F = WOFF_W00O + 4 * 64 * 256   # [2, 128, 256] stacked corner pairs
WOFF_W1 = WOFF_W00F + 2 * 128 * 256    # [2, 128, 256]
WOFF_W2 = WOFF_W1 + 2 * 128 * 256  # [2, 128, 1]
WBLOB = WOFF_W2 + 2 * 128 + 512    # pad to 205824 = 8 * 128 * 201
WSH = WBLOB // NCORES


# --------------------------------------------------------------------------
# host-side math (mirrors reference semantics in f32)
# --------------------------------------------------------------------------

def _corner_indices(co):
    """co: [N] f32 coords in one axis. Returns (base j in [0,65], i_minus,
    i_plus) exactly matching the reference's per-corner nearest indices."""
    # reference: c_t = clip(co + v/64 + eps, -1+1e-6, 1-1e-6);
    #            i_t = clip(round((c_t+1)*32 - 0.5), 0, 63)
    out = []
    for v in (-1.0, 1.0):
        c = np.clip(co + np.float32(v / 64.0) + EPS,
                    np.float32(-1 + 1e-6), np.float32(1 - 1e-6))
        i = np.clip(np.round((c + 1) * np.float32(32.0) - np.float32(0.5)),
                    0, 63).astype(np.int32)
        out.append(i)
    im, ip = out
    # padded-table base: j = clip(floor(ay), -1, 64) + 1, ay = 32*(co+eps)+31.5
    ay = (co + EPS) * np.float32(32.0) + np.float32(31.5)
    j = np.clip(np.floor(ay), -1, 64).astype(np.int32) + 1
    return j, im, ip


def _host_prep(inputs):
    feat = np.asarray(inputs['feat'], np.float32)
    inp = np.asarray(inputs['inp'], np.float32)
    coord = np.asarray(inputs['coord'], np.float32)
    cell = np.asarray(inputs['cell'], np.float32)
    scale = np.asarray(inputs['scale'], np.float32)
    Wq = np.asarray(inputs['Wq'], np.float32); bq = np.asarray(inputs['bq'], np.float32)
    Wk = np.asarray(inputs['Wk'], np.float32); bk = np.asarray(inputs['bk'], np.float32)
    Wv = np.asarray(inputs['Wv'], np.float32); bv = np.asarray(inputs['bv'], np.float32)
    W00 = np.asarray(inputs['W00'], np.float32); b00 = np.asarray(inputs['b00'], np.float32)
    W1 = np.asarray(inputs['W1'], np.float32); b1 = np.asarray(inputs['b1'], np.float32)
    W2 = np.asarray(inputs['W2'], np.float32); b2 = np.asarray(inputs['b2'], np.float32)
    ls = np.asarray(inputs['ls'], np.float32)

    # feat as bf16 rows [B, 4096, 64]: row iy*64+ix = feat[b, :, iy, ix]
    featrows = np.ascontiguousarray(
        feat.transpose(0, 2, 3, 1).reshape(B, NROW, C)).astype(BF16)

    coord_y = coord[..., 0].reshape(B, NPB)
    coord_x = coord[..., 1].reshape(B, NPB)

    # per-(b) padded-table base index; per-corner rel offsets + RBF weights
    idx_all = np.empty((B, NPB), np.int16)
    rel_all = np.empty((B, 4, 2, NPB), BF16)   # [rel_y, rel_x]
    w_all = np.empty((B, 4, NPB), BF16)
    hw = np.float32(64.0)
    ls2 = ls[0] * ls[0]
    for b in range(B):
        jy, iym, iyp = _corner_indices(coord_y[b])
        jx, ixm, ixp = _corner_indices(coord_x[b])
        idx_all[b] = (jy * np.int32(66) + jx).astype(np.int16)
        iy = {-1: iym, 1: iyp}
        ix = {-1: ixm, 1: ixp}
        t = 0
        for vx in (-1, 1):          # y offset
            for vy in (-1, 1):      # x offset
                oy = (iy[vx].astype(np.float32) + np.float32(0.5)) / np.float32(32.0) - 1
                ox = (ix[vy].astype(np.float32) + np.float32(0.5)) / np.float32(32.0) - 1
                ry = coord_y[b] - oy
                rx = coord_x[b] - ox
                rel_all[b, t, 0] = ry.astype(BF16)
                rel_all[b, t, 1] = rx.astype(BF16)
                rd = (ry * hw) ** 2 + (rx * hw) ** 2
                w_all[b, t] = np.exp(rd / ls2 * np.float32(-0.5)).astype(BF16)
                t += 1

    # ---- bilinear sample of inp (border, align_corners=False) + b2 ----
    bil = np.empty((B, NPB), BF16)
    for b in range(B):
        im = inp[b, 0]
        y = np.clip((coord_y[b] + 1) * np.float32(32.0) - np.float32(0.5), 0.0, 63.0)
        x = np.clip((coord_x[b] + 1) * np.float32(32.0) - np.float32(0.5), 0.0, 63.0)
        y0 = np.floor(y); x0 = np.floor(x)
        wy = (y - y0).astype(np.float32); wx = (x - x0).astype(np.float32)
        y0i = np.clip(y0.astype(np.int32), 0, 63)
        y1i = np.clip(y0.astype(np.int32) + 1, 0, 63)
        x0i = np.clip(x0.astype(np.int32), 0, 63)
        x1i = np.clip(x0.astype(np.int32) + 1, 0, 63)
        v00 = im[y0i, x0i]; v01 = im[y0i, x1i]
        v10 = im[y1i, x0i]; v11 = im[y1i, x1i]
        bil[b] = ((v00 * (1 - wy) * (1 - wx) + v01 * (1 - wy) * wx
                   + v10 * wy * (1 - wx) + v11 * wy * wx) + b2[0]).astype(BF16)

    # ---- rel -> int8 with the dequant scale folded into Wq's rel rows ----
    relmax = float(np.max(np.abs(rel_all.astype(np.float32)))) or 1.0
    QK = np.float32(127.0 / relmax)
    rel8 = np.clip(np.round(rel_all.astype(np.float32) * QK),
                   -127, 127).astype(np.int8)                               # [B,4,2,NPB]

    # ---- weight repacks ----
    wq_rhs = (Wq.T / QK).astype(BF16)                                       # [2, 64]
    wv_lhsT = Wv.T.astype(BF16)                                             # [64, 64]
    w00off_rhs = np.stack([W00[:, t * 64:(t + 1) * 64].T for t in range(4)]
                          ).astype(BF16)                                    # [4, 64, 256]
    # stacked corner-pair lhsT for the x1 fs-term: rows 0:64 = corner 2p,
    # rows 64:128 = corner 2p+1
    w00fp = np.stack(
        [np.concatenate([W00[:, 256 + 2 * p * 64: 256 + (2 * p + 1) * 64].T,
                         W00[:, 256 + (2 * p + 1) * 64: 256 + (2 * p + 2) * 64].T],
                        axis=0) for p in range(2)]).astype(BF16)            # [2, 128, 256]
    # fold the scalar grid tail (b00 + W00[:,512:516] @ [cell*hw, scale])
    # through W1 into the gelu bias: b1eff = b1 + W1 @ b00eff
    b1eff = np.empty((B, 2, 128, 1), np.float32)
    for b in range(B):
        vec4 = np.concatenate([cell[b] * hw, scale[b]]).astype(np.float32)
        b00eff = b00 + W00[:, 512:516] @ vec4
        b1eff[b] = (b1 + W1 @ b00eff).reshape(2, 128, 1)
    w1_lhsT = np.ascontiguousarray(W1.T.astype(BF16).reshape(2, 128, 256))  # [2, 128, 256]
    w2_lhsT = np.ascontiguousarray(W2.T.astype(BF16).reshape(2, 128, 1))    # [2, 128, 1]

    # ---- bf16 weight blob (AllGathered on device): flat row-major concat ----
    wflat = np.concatenate([
        wq_rhs.reshape(-1), bq.astype(BF16), Wk.T.astype(BF16).reshape(-1),
        bk.astype(BF16), wv_lhsT.reshape(-1),
        w00off_rhs.reshape(-1), w00fp.reshape(-1),
        w1_lhsT.reshape(-1), w2_lhsT.reshape(-1),
        np.zeros(512, BF16)])
    assert wflat.size == WBLOB, wflat.size

    # ---- shard per core ----
    NFS = NROW // NCORES     # 512 feat rows per core shard (AllGathered on device)
    in_maps = []
    for cidx in range(NCORES):
        sl = slice(cidx * NLOC, (cidx + 1) * NLOC)
        # pixel-major tiles: local pixel j*128+p at [p, j]; each gathered
        # table row holds all 4 corners (c00|c01|c10|c11), so wsm is laid
        # out corner-minor [p, j*4+t] to broadcast-multiply the row.
        idx2d = np.ascontiguousarray(
            idx_all[:, sl].reshape(B, 64, 128).transpose(0, 2, 1))
        wsm2d = np.ascontiguousarray(
            w_all[:, :, sl].reshape(B, 4, 64, 128).transpose(0, 3, 2, 1)
            .reshape(B, 128, 4 * 64))
        m = {
            'feati': np.ascontiguousarray(
                featrows[:, cidx * NFS:(cidx + 1) * NFS, :]).reshape(B, 128, 256),
            'wblob': np.ascontiguousarray(
                wflat[cidx * WSH:(cidx + 1) * WSH]).reshape(128, WSH // 128),
            'idx': idx2d,
            'wsm': wsm2d,
            'relq': np.ascontiguousarray(rel8[:, :, :, sl]).reshape(B, 8, NLOC),
            'bil': np.ascontiguousarray(bil[:, sl]),
            'bv': np.concatenate([bv, bv]).reshape(128, 1).astype(np.float32),
            'b1': b1eff,
        }
        in_maps.append(m)
    return in_maps


# --------------------------------------------------------------------------
# device kernel
# --------------------------------------------------------------------------

@functools.lru_cache(maxsize=4)
def _build(qk_bias=False):
    import concourse.bass as bass
    import concourse.tile as tile
    from concourse import bacc, mybir
    dt = mybir.dt
    F32, BF, I16 = dt.float32, dt.bfloat16, dt.int16
    AF = mybir.ActivationFunctionType
    ALU = mybir.AluOpType

    nc = bacc.Bacc(None, target_bir_lowering=False)

    feati = nc.dram_tensor('feati', [B, 128, 256], BF, kind='ExternalInput')
    wblob = nc.dram_tensor('wblob', [128, WSH // 128], BF, kind='ExternalInput')
    idx = nc.dram_tensor('idx', [B, 128, 64], I16, kind='ExternalInput')
    wsm = nc.dram_tensor('wsm', [B, 128, 4 * 64], BF, kind='ExternalInput')
    relq = nc.dram_tensor('relq', [B, 8, NLOC], dt.int8, kind='ExternalInput')
    bil = nc.dram_tensor('bil', [B, NLOC], BF, kind='ExternalInput')
    bv = nc.dram_tensor('bv', [128, 1], F32, kind='ExternalInput')
    b1 = nc.dram_tensor('b1', [B, 2, 128, 1], F32, kind='ExternalInput')
    out = nc.dram_tensor('out', [B, NLOC], BF, kind='ExternalOutput')

    NU = B * 4  # 8 attention units

    with tile.TileContext(nc) as tc:
        with (
            tc.tile_pool(name='const', bufs=1) as constp,
            tc.tile_pool(name='fs', bufs=1) as fsp,
            tc.tile_pool(name='gat', bufs=1) as gatp,
            tc.tile_pool(name='qk', bufs=1) as qkp,
            tc.tile_pool(name='rel', bufs=1) as relp,
            tc.tile_pool(name='v', bufs=1) as vp,
            tc.tile_pool(name='mlp', bufs=1) as mlpp,
            tc.tile_pool(name='small', bufs=1) as smallp,
            tc.tile_pool(name='ps', bufs=1, space='PSUM') as psp,
            tc.tile_pool(name='psx', bufs=1, space='PSUM') as psxp,
            tc.tile_pool(name='dram', bufs=1, space='DRAM') as dramp,
        ):
            # ---- AllGather feat row shards and the weight blob ----
            featfull = [dramp.tile([NROW, C], BF, name=f'featfull{_b}')
                        for _b in range(B)]
            for _b in range(B):
                ccf_in = dramp.tile([128, 256], BF, name=f'ccf_in{_b}')
                nc.sync.dma_start(out=ccf_in[:], in_=feati[_b, :, :])
                nc.gpsimd.collective_compute(
                    'AllGather', mybir.AluOpType.bypass,
                    replica_groups=[list(range(NCORES))],
                    ins=[ccf_in.opt()], outs=[featfull[_b].opt()],
                )
            wfull = dramp.tile([WBLOB], BF, name='wfull')
            wcc_in = dramp.tile([128, WSH // 128], BF, name='wcc_in')
            nc.sync.dma_start(out=wcc_in[:], in_=wblob[:, :])
            nc.gpsimd.collective_compute(
                'AllGather', mybir.AluOpType.bypass,
                replica_groups=[list(range(NCORES))],
                ins=[wcc_in.opt()], outs=[wfull.opt()],
            )

            # ---- 66x66 edge-replicated 2x2-patch table, built on device ----
            # ptable[b][jy*66+jx] = [c00|c01|c10|c11],
            # c(dy,dx) = feat[b, :, clip(jy-1+dy,0,63), clip(jx-1+dx,0,63)]
            NTAB = 66 * 66
            ptable = [dramp.tile([NTAB, 256], BF, name=f'ptable{_b}')
                      for _b in range(B)]
            for _b in range(B):
                pt_t = ptable[_b][:, :].tensor
                ff_t = featfull[_b][:, :].tensor
                for dy in (0, 1):
                    yr = ([(0, 1, 0), (1, 64, 0), (65, 1, 63)] if dy == 0
                          else [(0, 64, 0), (64, 2, 63)])
                    for dx in (0, 1):
                        xr = ([(0, 1, 0), (1, 64, 0), (65, 1, 63)] if dx == 0
                              else [(0, 64, 0), (64, 2, 63)])
                        qoff = (dy * 2 + dx) * 64
                        for ri, (jy0, ny, sy0) in enumerate(yr):
                            for (jx0, nx, sx0) in xr:
                                dst = bass.AP(
                                    pt_t, (jy0 * 66 + jx0) * 256 + qoff,
                                    [(66 * 256, ny), (256, nx), (1, 64)])
                                src = bass.AP(
                                    ff_t, (sy0 * 64 + sx0) * 64,
                                    [(4096 if ny > 1 and sy0 == 0 else 0, ny),
                                     (64 if nx > 1 and sx0 == 0 else 0, nx),
                                     (1, 64)])
                                # split the build across two DMA queues
                                eng = nc.sync if (dy * 2 + dx + ri) % 2 else nc.gpsimd
                                eng.dma_start(out=dst, in_=src)

            # ---- constant weights to SBUF (from the gathered blob) ----
            # Per corner-pair slot p: Wq.T/QK at rows 2t:2t+2, cols tt*64, so
            # one matmul with the full [8, .] rel tile as lhsT yields
            # [q_{2p} | q_{2p+1}]; wk is block-diagonal for the same pairing.
            wq_sb = constp.tile([8, 2, 128], BF)
            bq_sb = constp.tile([1, 128], BF)
            wk_sb = constp.tile([128, 128], BF)
            bk_sb = constp.tile([1, 128], BF)
            wv_sb = constp.tile([128, 64], BF)   # Wv.T duplicated in both halves
            bv_sb = constp.tile([128, 1], F32)
            w00o_sb = constp.tile([64, 4 * 256], BF)
            w00fp_sb = constp.tile([128, 2, 256], BF)
            w1_sb = constp.tile([128, 2, 256], BF)
            b1_sb = constp.tile([128, B, 2], F32)
            w2_sb = constp.tile([128, 2], BF)
            nc.vector.memset(wq_sb[:], 0.0)
            nc.vector.memset(wk_sb[:], 0.0)
            for _p in range(2):
                for _tt in range(2):
                    _t = 2 * _p + _tt
                    nc.sync.dma_start(
                        out=wq_sb[2 * _t:2 * _t + 2, _p, _tt * 64:(_tt + 1) * 64],
                        in_=wfull[WOFF_WQ:WOFF_BQ])
                nc.sync.dma_start(out=bq_sb[:, _p * 64:(_p + 1) * 64],
                                  in_=wfull[WOFF_BQ:WOFF_WK])
                nc.sync.dma_start(
                    out=wk_sb[_p * 64:(_p + 1) * 64, _p * 64:(_p + 1) * 64],
                    in_=wfull[WOFF_WK:WOFF_BK])
                nc.sync.dma_start(out=bk_sb[:, _p * 64:(_p + 1) * 64],
                                  in_=wfull[WOFF_BK:WOFF_WV])
            nc.sync.dma_start(out=wv_sb[0:64, :], in_=wfull[WOFF_WV:WOFF_W00O])
            nc.sync.dma_start(out=wv_sb[64:128, :], in_=wfull[WOFF_WV:WOFF_W00O])
            nc.sync.dma_start(out=bv_sb[:], in_=bv[:, :])
            for t in range(4):
                nc.sync.dma_start(
                    out=w00o_sb[:, t * 256:(t + 1) * 256],
                    in_=wfull[WOFF_W00O + t * 16384:WOFF_W00O + (t + 1) * 16384])
            for kk in range(2):
                nc.sync.dma_start(
                    out=w00fp_sb[:, kk, :],
                    in_=wfull[WOFF_W00F + kk * 32768:WOFF_W00F + (kk + 1) * 32768])
                nc.sync.dma_start(
                    out=w1_sb[:, kk, :],
                    in_=wfull[WOFF_W1 + kk * 32768:WOFF_W1 + (kk + 1) * 32768])
                for _b in range(B):
                    nc.sync.dma_start(out=b1_sb[:, _b, kk:kk + 1],
                                      in_=b1[_b, kk, :, :])
                nc.sync.dma_start(
                    out=w2_sb[:, kk:kk + 1],
                    in_=wfull[WOFF_W2 + kk * 128:WOFF_W2 + (kk + 1) * 128])

            Sp_sb = constp.tile([64, NU * 64], F32)   # partial logits, all units

            # =========== phases 1+2 per batch: gather, fs, q/k, S ===========
            from concourse.masks import make_identity
            ident_sb = constp.tile([128, 128], BF)
            make_identity(nc, ident_sb[:])


            ones_nl = constp.tile([1, NLOC], BF)
            nc.vector.memset(ones_nl[:], 1.0)

            # Per batch: quarters of 16 pixel-tiles stream through gather ->
            # RBF multiply -> [128,128] pair transposes -> fs pair chunks,
            # and each quarter's q/k matmuls + logit accumulation run right
            # behind it so the PE overlaps the gather instead of idling.
            fsp_all = [[fsp.tile([128, NLOC], BF, name=f'fsp{_b}_{_p}')
                        for _p in range(2)] for _b in range(B)]
            for b in range(B):
                idx16_sb = gatp.tile([128, 64], I16)
                wsm_sb = gatp.tile([128, 4 * 64], BF)
                idx_sb = gatp.tile([128, 64], dt.int32)
                nc.sync.dma_start(out=idx16_sb[:], in_=idx[b, :, :])
                nc.sync.dma_start(out=wsm_sb[:], in_=wsm[b, :, :])
                nc.vector.tensor_copy(out=idx_sb[:], in_=idx16_sb[:])
                rel8_sb = relp.tile([8, NLOC], dt.int8, name='rel8')
                nc.sync.dma_start(out=rel8_sb[:], in_=relq[b, :, :])
                rel_sb = relp.tile([8, NLOC], BF)
                nc.vector.tensor_copy(out=rel_sb[:], in_=rel8_sb[:])

                s_all = psp.tile([64, 4 * 64], F32, name='s_all')
                for q in range(4):
                    g_pm = gatp.tile([128, 16, 4 * C], BF, name=f'g_pm{q % 2}')
                    for o in range(16):
                        nc.gpsimd.indirect_dma_start(
                            out=g_pm[:, o, :], out_offset=None,
                            in_=ptable[b][:, :],
                            in_offset=bass.IndirectOffsetOnAxis(
                                ap=idx_sb[:, q * 16 + o:q * 16 + o + 1], axis=0))
                    wap = wsm_sb[:, q * 64:(q + 1) * 64]
                    wbc = bass.AP(wap.tensor, wap.offset, wap.ap + [(0, C)])
                    nc.vector.tensor_tensor(out=g_pm[:, :, :],
                                            in0=g_pm[:, :, :], in1=wbc,
                                            op=ALU.mult)
                    for p in range(2):
                        for jg in range(4):
                            tp_ps = psp.tile([128, 512], BF, name='tp')
                            for jj in range(4):
                                jl = jg * 4 + jj
                                nc.tensor.transpose(
                                    out=tp_ps[:, jj * 128:(jj + 1) * 128],
                                    in_=g_pm[:, jl, p * 128:(p + 1) * 128],
                                    identity=ident_sb[:])
                            goff = (q * 16 + jg * 4) * 128
                            nc.scalar.copy(
                                out=fsp_all[b][p][:, goff:goff + 512],
                                in_=tp_ps[:])
                    # q/k + logit accumulation over this quarter's pixels;
                    # one matmul per pixel-tile covers both corners of a pair
                    for p in range(2):
                        fpt = fsp_all[b][p]
                        for jg in range(4):      # 4 groups of 4 pixel-tiles
                            q_ps = psp.tile([128, 512], F32)
                            k_ps = psp.tile([128, 512], F32)
                            for jj in range(4):
                                j = q * 16 + jg * 4 + jj
                                csl = slice(j * 128, (j + 1) * 128)
                                osl = slice(jj * 128, (jj + 1) * 128)
                                nc.tensor.matmul(
                                    out=q_ps[:, osl],
                                    lhsT=rel_sb[:, csl],
                                    rhs=wq_sb[:, p, :],
                                    start=True, stop=not qk_bias)
                                nc.tensor.matmul(
                                    out=k_ps[:, osl], lhsT=fpt[:, csl],
                                    rhs=wk_sb[:, :],
                                    start=True, stop=not qk_bias)
                                if qk_bias:
                                    nc.tensor.matmul(
                                        out=q_ps[:, osl], lhsT=ones_nl[:, csl],
                                        rhs=bq_sb[:], start=False, stop=True)
                                    nc.tensor.matmul(
                                        out=k_ps[:, osl], lhsT=ones_nl[:, csl],
                                        rhs=bk_sb[:], start=False, stop=True)
                            qs_sb = qkp.tile([128, 512], BF, name=f'qs{jg % 2}')
                            ks_sb = qkp.tile([128, 512], BF, name=f'ks{jg % 2}')
                            nc.scalar.activation(out=qs_sb[:], in_=q_ps[:], func=AF.Relu)
                            nc.vector.tensor_scalar_max(out=ks_sb[:], in0=k_ps[:], scalar1=0.0)
                            for jj in range(4):
                                for tt in range(2):
                                    t = 2 * p + tt
                                    ssl = slice(jj * 128 + tt * 64,
                                                jj * 128 + (tt + 1) * 64)
                                    nc.tensor.matmul(
                                        out=s_all[:, t * 64:(t + 1) * 64],
                                        lhsT=qs_sb[:, ssl],
                                        rhs=ks_sb[:, ssl],
                                        start=(q == 0 and jg == 0 and jj == 0),
                                        stop=(q == 3 and jg == 3 and jj == 3))
                nc.vector.tensor_copy(out=Sp_sb[:, b * 256:(b + 1) * 256],
                                      in_=s_all[:])

            # =========== phase 3: AllReduce of logits ===========
            cc_in = dramp.tile([64, NU * 64], F32)
            cc_out = dramp.tile([64, NU * 64], F32)
            nc.gpsimd.dma_start(out=cc_in[:], in_=Sp_sb[:])
            nc.gpsimd.collective_compute(
                'AllReduce', mybir.AluOpType.add,
                replica_groups=[list(range(NCORES))],
                ins=[cc_in.opt()], outs=[cc_out.opt()],
            )
            S_sb = constp.tile([64, NU * 64], F32)
            nc.gpsimd.dma_start(out=S_sb[:], in_=cc_out[:])

            # =========== phase 4: softmax + A_t^T ===========
            attn_sb = constp.tile([64, NU * 64], BF)
            AT_tiles = []
            for u in range(NU):
                usl = slice(u * 64, (u + 1) * 64)
                mx = smallp.tile([64, 1], F32)
                nmx = smallp.tile([64, 1], F32)
                ex = smallp.tile([64, 64], F32)
                sm = smallp.tile([64, 1], F32)
                rs = smallp.tile([64, 1], F32)
                nc.vector.tensor_reduce(out=mx[:], in_=S_sb[:, usl],
                                        axis=mybir.AxisListType.X, op=ALU.max)
                nc.vector.tensor_scalar_mul(out=nmx[:], in0=mx[:], scalar1=-1.0)
                nc.scalar.activation(out=ex[:], in_=S_sb[:, usl], func=AF.Exp,
                                     bias=nmx[:, 0:1])
                nc.vector.tensor_reduce(out=sm[:], in_=ex[:],
                                        axis=mybir.AxisListType.X, op=ALU.add)
                nc.vector.reciprocal(out=rs[:], in_=sm[:])
                nc.vector.tensor_scalar_mul(out=attn_sb[:, usl], in0=ex[:],
                                            scalar1=rs[:, 0:1])
            # v = relu(Wv fs + bv) is independent of the attention logits:
            # queue the first super-chunk's v for both batches ahead of the
            # AT matmuls so the PE isn't stalled behind the AllReduce, then
            # prefetch v(pc+1) ahead of each MLP chunk.
            def compute_v(b, pc):
                tiles = []
                for p in range(2):
                    vt = vp.tile([128, PCH], BF, name=f'vt{b}_{p}_{pc % 2}')
                    for cc in range(PCH // CHUNK):
                        vsl_l = slice(cc * CHUNK, (cc + 1) * CHUNK)
                        vsl_g = slice(pc * PCH + cc * CHUNK,
                                      pc * PCH + (cc + 1) * CHUNK)
                        v_ps = psp.tile([128, CHUNK], F32)
                        nc.tensor.matmul(out=v_ps[0:64, :], lhsT=wv_sb[0:64, :],
                                         rhs=fsp_all[b][p][0:64, vsl_g],
                                         start=True, stop=True)
                        nc.tensor.matmul(out=v_ps[64:128, :], lhsT=wv_sb[64:128, :],
                                         rhs=fsp_all[b][p][64:128, vsl_g],
                                         start=True, stop=True)
                        nc.scalar.activation(out=vt[:, vsl_l], in_=v_ps[:],
                                             func=AF.Relu, bias=bv_sb[:, 0:1])
                    tiles.append(vt)
                return tiles

            v_next = {_b: compute_v(_b, 0) for _b in range(B)}
            v_next2 = {_b: compute_v(_b, 1) for _b in range(B)}

            for b in range(B):
                for p in range(2):
                    a_full = psp.tile([128, 512], F32, name='misc_ps')
                    for h in range(2):
                        t = 2 * p + h
                        u = b * 4 + t
                        nc.tensor.matmul(
                            out=a_full[h * 64:(h + 1) * 64, 0:256],
                            lhsT=attn_sb[:, u * 64:(u + 1) * 64],
                            rhs=w00o_sb[:, t * 256:(t + 1) * 256],
                            start=True, stop=True)
                    at = constp.tile([128, 256], BF, name=f'atp{b}_{p}')
                    nc.vector.tensor_copy(out=at[:], in_=a_full[:, 0:256])
                    AT_tiles.append(at)

            # =========== phase 5: MLP over resident fs pairs ===========
            for b in range(B):
                bil_sb = smallp.tile([1, NLOC], BF, name='bil_sb')
                nc.sync.dma_start(out=bil_sb[:], in_=bil[b, :][None, :])
                o_row = smallp.tile([1, NLOC], BF, name='o_row')

                for pc in range(NLOC // PCH):
                    v_tiles = v_next[b]
                    v_next[b] = v_next2[b]
                    if pc + 2 < NLOC // PCH:
                        v_next2[b] = compute_v(b, pc + 2)

                    x1_t = [mlpp.tile([128, PCH], BF, name=f'x1_{_m}_{pc % 2}')
                            for _m in range(2)]
                    x2_t = [mlpp.tile([128, PCH], BF, name=f'x2_{_m}_{pc % 2}')
                            for _m in range(2)]
                    for cc in range(PCH // CHUNK):
                        lsl = slice(cc * CHUNK, (cc + 1) * CHUNK)
                        gsl = slice(pc * PCH + cc * CHUNK, pc * PCH + (cc + 1) * CHUNK)
                        for m in range(2):
                            msl = slice(m * 128, (m + 1) * 128)
                            x_ps = psxp.tile([128, CHUNK], F32)
                            for p in range(2):
                                nc.tensor.matmul(
                                    out=x_ps[:],
                                    lhsT=w00fp_sb[:, p, msl],
                                    rhs=fsp_all[b][p][:, gsl],
                                    start=(p == 0), stop=False)
                            for p in range(2):
                                at = AT_tiles[b * 2 + p]
                                nc.tensor.matmul(
                                    out=x_ps[:],
                                    lhsT=at[:, msl],
                                    rhs=v_tiles[p][:, lsl],
                                    start=False, stop=(p == 1))
                            nc.vector.tensor_copy(out=x1_t[m][:, lsl], in_=x_ps[:])
                        # W1 + gelu
                        for m in range(2):
                            msl = slice(m * 128, (m + 1) * 128)
                            x2_ps = psxp.tile([128, CHUNK], F32)
                            for kk in range(2):
                                nc.tensor.matmul(out=x2_ps[:],
                                                 lhsT=w1_sb[:, kk, msl],
                                                 rhs=x1_t[kk][:, lsl],
                                                 start=(kk == 0), stop=(kk == 1))
                            nc.scalar.activation(out=x2_t[m][:, lsl], in_=x2_ps[:],
                                                 func=AF.Gelu, bias=b1_sb[:, b, m:m + 1])
                        # W2; bil add batched once per batch below
                        o_full = psp.tile([64, 512], F32, name='misc_ps')
                        o_ps = o_full[0:1, :]
                        for kk in range(2):
                            nc.tensor.matmul(out=o_ps, lhsT=w2_sb[:, kk:kk + 1],
                                             rhs=x2_t[kk][:, lsl],
                                             start=(kk == 0), stop=(kk == 1))
                        nc.scalar.copy(out=o_row[:, gsl], in_=o_ps)
                nc.vector.tensor_tensor(out=o_row[:], in0=o_row[:],
                                        in1=bil_sb[:], op=ALU.add)
                nc.sync.dma_start(out=out[b, :][None, :], in_=o_row[:])

    nc.compile()
    return nc


# --------------------------------------------------------------------------

def kernel(**inputs) -> np.ndarray:
    from concourse.bass_utils import run_bass_kernel_spmd
    in_maps = _host_prep(inputs)
    qk_bias = bool(np.any(np.asarray(inputs['bq']))
                   or np.any(np.asarray(inputs['bk'])))
    nc = _build(qk_bias)
    res = run_bass_kernel_spmd(nc, in_maps, core_ids=list(range(NCORES)))
    full = np.empty((B, 1, HQ, WQ), np.float32)
    flat = full.reshape(B, NPB)
    for cidx in range(NCORES):
        flat[:, cidx * NLOC:(cidx + 1) * NLOC] = \
            res.results[cidx]['out'].astype(np.float32)
    return full


# revision 95
# speedup vs baseline: 1.0055x; 1.0055x over previous
"""Trainium2 Bass kernel for nn_AnyTSRpp (sparse_attention).

Strategy: pure data-parallel over the HR pixel grid (65536 px/batch),
8192 px/batch/core on 8 NeuronCores. Host computes per-pixel corner
indices/scalars; device gathers feat rows directly (per-corner indirect
DMA, pixel-major), applies the RBF weight per-partition pre-transpose,
PE transposes to channel-major, runs all matmuls/relu/softmax/gelu, and
a tiny AllReduce for the global attention logits (contraction over all
pixels). off_t = attn_t @ v_t is folded as (W00_off_t @ attn_t) @ v_t
so the attention output is never materialized.

Self-contained: hardcodes all shapes. kernel(**inputs) -> np.ndarray.
"""

import functools
import numpy as np
import ml_dtypes

BF16 = ml_dtypes.bfloat16


def _setup_jax_cache():
    """Persistent XLA compilation cache: repeated/every-process calls skip
    the neuronx-cc recompile of the identical kernel graph."""
    import jax
    try:
        jax.config.update('jax_compilation_cache_dir', '/root/.cache/jax_pcache')
        jax.config.update('jax_persistent_cache_min_compile_time_secs', 0.0)
        jax.config.update('jax_persistent_cache_min_entry_size_bytes', 0)
    except Exception:
        pass


_setup_jax_cache()

NCORES = 8
B = 2
C = 64
HLR = WLR = 64
HQ = WQ = 256
NPB = HQ * WQ            # 65536 pixels per batch
NLOC = NPB // NCORES     # 8192 pixels per batch per core
NROW = HLR * WLR         # 4096 feat rows (y-major)
CHUNK = 512              # matmul moving-N chunk
NCHUNK = NLOC // CHUNK   # 16
PCH = 512                # MLP pixel super-chunk
EPS = np.float32(1e-6)

# bf16 weight blob layout (flat element offsets)
WOFF_WQ = 0                        # [2, 64]  Wq.T/QK
WOFF_BQ = WOFF_WQ + 2 * 64         # [1, 64]
WOFF_WK = WOFF_BQ + 64             # [64, 64] Wk.T
WOFF_BK = WOFF_WK + 64 * 64        # [1, 64]
WOFF_WV = WOFF_BK + 64             # [64, 64]
WOFF_W00O = WOFF_WV + 64 * 64      # [4, 64, 256]
WOFF_W00F = WOFF_W00O + 4 * 64 * 256   # [2, 128, 256] stacked corner pairs
WOFF_W1 = WOFF_W00F + 2 * 128 * 256    # [2, 128, 256]
WOFF_W2 = WOFF_W1 + 2 * 128 * 256  # [2, 128, 1]
WBLOB = WOFF_W2 + 2 * 128 + 512    # pad to 205824 = 8 * 128 * 201
WSH = WBLOB // NCORES


# --------------------------------------------------------------------------
# host-side math (mirrors reference semantics in f32)
# --------------------------------------------------------------------------

def _corner_indices(co):
    """co: [N] f32 coords in one axis. Returns (base j in [0,65], i_minus,
    i_plus) exactly matching the reference's per-corner nearest indices."""
    # reference: c_t = clip(co + v/64 + eps, -1+1e-6, 1-1e-6);
    #            i_t = clip(round((c_t+1)*32 - 0.5), 0, 63)
    out = []
    for v in (-1.0, 1.0):
        c = np.clip(co + np.float32(v / 64.0) + EPS,
                    np.float32(-1 + 1e-6), np.float32(1 - 1e-6))
        i = np.clip(np.round((c + 1) * np.float32(32.0) - np.float32(0.5)),
                    0, 63).astype(np.int32)
        out.append(i)
    im, ip = out
    # padded-table base: j = clip(floor(ay), -1, 64) + 1, ay = 32*(co+eps)+31.5
    ay = (co + EPS) * np.float32(32.0) + np.float32(31.5)
    j = np.clip(np.floor(ay), -1, 64).astype(np.int32) + 1
    return j, im, ip


def _host_prep(inputs):
    feat = np.asarray(inputs['feat'], np.float32)
    inp = np.asarray(inputs['inp'], np.float32)
    coord = np.asarray(inputs['coord'], np.float32)
    cell = np.asarray(inputs['cell'], np.float32)
    scale = np.asarray(inputs['scale'], np.float32)
    Wq = np.asarray(inputs['Wq'], np.float32); bq = np.asarray(inputs['bq'], np.float32)
    Wk = np.asarray(inputs['Wk'], np.float32); bk = np.asarray(inputs['bk'], np.float32)
    Wv = np.asarray(inputs['Wv'], np.float32); bv = np.asarray(inputs['bv'], np.float32)
    W00 = np.asarray(inputs['W00'], np.float32); b00 = np.asarray(inputs['b00'], np.float32)
    W1 = np.asarray(inputs['W1'], np.float32); b1 = np.asarray(inputs['b1'], np.float32)
    W2 = np.asarray(inputs['W2'], np.float32); b2 = np.asarray(inputs['b2'], np.float32)
    ls = np.asarray(inputs['ls'], np.float32)

    # feat as bf16 rows [B, 4096, 64]: row iy*64+ix = feat[b, :, iy, ix]
    featrows = np.ascontiguousarray(
        feat.transpose(0, 2, 3, 1).reshape(B, NROW, C)).astype(BF16)

    coord_y = coord[..., 0].reshape(B, NPB)
    coord_x = coord[..., 1].reshape(B, NPB)

    # per-(b) padded-table base index; per-corner rel offsets + RBF weights
    idx_all = np.empty((B, NPB), np.int16)
    rel_all = np.empty((B, 4, 2, NPB), BF16)   # [rel_y, rel_x]
    w_all = np.empty((B, 4, NPB), BF16)
    hw = np.float32(64.0)
    ls2 = ls[0] * ls[0]
    for b in range(B):
        jy, iym, iyp = _corner_indices(coord_y[b])
        jx, ixm, ixp = _corner_indices(coord_x[b])
        idx_all[b] = (jy * np.int32(66) + jx).astype(np.int16)
        iy = {-1: iym, 1: iyp}
        ix = {-1: ixm, 1: ixp}
        t = 0
        for vx in (-1, 1):          # y offset
            for vy in (-1, 1):      # x offset
                oy = (iy[vx].astype(np.float32) + np.float32(0.5)) / np.float32(32.0) - 1
                ox = (ix[vy].astype(np.float32) + np.float32(0.5)) / np.float32(32.0) - 1
                ry = coord_y[b] - oy
                rx = coord_x[b] - ox
                rel_all[b, t, 0] = ry.astype(BF16)
                rel_all[b, t, 1] = rx.astype(BF16)
                rd = (ry * hw) ** 2 + (rx * hw) ** 2
                w_all[b, t] = np.exp(rd / ls2 * np.float32(-0.5)).astype(BF16)
                t += 1

    # ---- bilinear sample of inp (border, align_corners=False) + b2 ----
    bil = np.empty((B, NPB), BF16)
    for b in range(B):
        im = inp[b, 0]
        y = np.clip((coord_y[b] + 1) * np.float32(32.0) - np.float32(0.5), 0.0, 63.0)
        x = np.clip((coord_x[b] + 1) * np.float32(32.0) - np.float32(0.5), 0.0, 63.0)
        y0 = np.floor(y); x0 = np.floor(x)
        wy = (y - y0).astype(np.float32); wx = (x - x0).astype(np.float32)
        y0i = np.clip(y0.astype(np.int32), 0, 63)
        y1i = np.clip(y0.astype(np.int32) + 1, 0, 63)
        x0i = np.clip(x0.astype(np.int32), 0, 63)
        x1i = np.clip(x0.astype(np.int32) + 1, 0, 63)
        v00 = im[y0i, x0i]; v01 = im[y0i, x1i]
        v10 = im[y1i, x0i]; v11 = im[y1i, x1i]
        bil[b] = ((v00 * (1 - wy) * (1 - wx) + v01 * (1 - wy) * wx
                   + v10 * wy * (1 - wx) + v11 * wy * wx) + b2[0]).astype(BF16)

    # ---- rel -> int8 with the dequant scale folded into Wq's rel rows ----
    relmax = float(np.max(np.abs(rel_all.astype(np.float32)))) or 1.0
    QK = np.float32(127.0 / relmax)
    rel8 = np.clip(np.round(rel_all.astype(np.float32) * QK),
                   -127, 127).astype(np.int8)                               # [B,4,2,NPB]

    # ---- weight repacks ----
    wq_rhs = (Wq.T / QK).astype(BF16)                                       # [2, 64]
    wv_lhsT = Wv.T.astype(BF16)                                             # [64, 64]
    w00off_rhs = np.stack([W00[:, t * 64:(t + 1) * 64].T for t in range(4)]
                          ).astype(BF16)                                    # [4, 64, 256]
    # stacked corner-pair lhsT for the x1 fs-term: rows 0:64 = corner 2p,
    # rows 64:128 = corner 2p+1
    w00fp = np.stack(
        [np.concatenate([W00[:, 256 + 2 * p * 64: 256 + (2 * p + 1) * 64].T,
                         W00[:, 256 + (2 * p + 1) * 64: 256 + (2 * p + 2) * 64].T],
                        axis=0) for p in range(2)]).astype(BF16)            # [2, 128, 256]
    # fold the scalar grid tail (b00 + W00[:,512:516] @ [cell*hw, scale])
    # through W1 into the gelu bias: b1eff = b1 + W1 @ b00eff
    b1eff = np.empty((B, 2, 128, 1), np.float32)
    for b in range(B):
        vec4 = np.concatenate([cell[b] * hw, scale[b]]).astype(np.float32)
        b00eff = b00 + W00[:, 512:516] @ vec4
        b1eff[b] = (b1 + W1 @ b00eff).reshape(2, 128, 1)
    w1_lhsT = np.ascontiguousarray(W1.T.astype(BF16).reshape(2, 128, 256))  # [2, 128, 256]
    w2_lhsT = np.ascontiguousarray(W2.T.astype(BF16).reshape(2, 128, 1))    # [2, 128, 1]

    # ---- bf16 weight blob (AllGathered on device): flat row-major concat ----
    wflat = np.concatenate([
        wq_rhs.reshape(-1), bq.astype(BF16), Wk.T.astype(BF16).reshape(-1),
        bk.astype(BF16), wv_lhsT.reshape(-1),
        w00off_rhs.reshape(-1), w00fp.reshape(-1),
        w1_lhsT.reshape(-1), w2_lhsT.reshape(-1),
        np.zeros(512, BF16)])
    assert wflat.size == WBLOB, wflat.size

    # ---- shard per core ----
    NFS = NROW // NCORES     # 512 feat rows per core shard (AllGathered on device)
    in_maps = []
    for cidx in range(NCORES):
        sl = slice(cidx * NLOC, (cidx + 1) * NLOC)
        # pixel-major tiles: local pixel j*128+p at [p, j]; each gathered
        # table row holds all 4 corners (c00|c01|c10|c11), so wsm is laid
        # out corner-minor [p, j*4+t] to broadcast-multiply the row.
        idx2d = np.ascontiguousarray(
            idx_all[:, sl].reshape(B, 64, 128).transpose(0, 2, 1))
        wsm2d = np.ascontiguousarray(
            w_all[:, :, sl].reshape(B, 4, 64, 128).transpose(0, 3, 2, 1)
            .reshape(B, 128, 4 * 64))
        m = {
            'feati': np.ascontiguousarray(
                featrows[:, cidx * NFS:(cidx + 1) * NFS, :]).reshape(B, 128, 256),
            'wblob': np.ascontiguousarray(
                wflat[cidx * WSH:(cidx + 1) * WSH]).reshape(128, WSH // 128),
            'idx': idx2d,
            'wsm': wsm2d,
            'relq': np.ascontiguousarray(rel8[:, :, :, sl]).reshape(B, 8, NLOC),
            'bil': np.ascontiguousarray(bil[:, sl]),
            'bv': np.concatenate([bv, bv]).reshape(128, 1).astype(np.float32),
            'b1': b1eff,
        }
        in_maps.append(m)
    return in_maps


# --------------------------------------------------------------------------
# device kernel
# --------------------------------------------------------------------------

@functools.lru_cache(maxsize=4)
def _build(qk_bias=False):
    import concourse.bass as bass
    import concourse.tile as tile
    from concourse import bacc, mybir
    dt = mybir.dt
    F32, BF, I16 = dt.float32, dt.bfloat16, dt.int16
    AF = mybir.ActivationFunctionType
    ALU = mybir.AluOpType

    nc = bacc.Bacc(None, target_bir_lowering=False)

    feati = nc.dram_tensor('feati', [B, 128, 256], BF, kind='ExternalInput')
    wblob = nc.dram_tensor('wblob', [128, WSH // 128], BF, kind='ExternalInput')
    idx = nc.dram_tensor('idx', [B, 128, 64], I16, kind='ExternalInput')
    wsm = nc.dram_tensor('wsm', [B, 128, 4 * 64], BF, kind='ExternalInput')
    relq = nc.dram_tensor('relq', [B, 8, NLOC], dt.int8, kind='ExternalInput')
    bil = nc.dram_tensor('bil', [B, NLOC], BF, kind='ExternalInput')
    bv = nc.dram_tensor('bv', [128, 1], F32, kind='ExternalInput')
    b1 = nc.dram_tensor('b1', [B, 2, 128, 1], F32, kind='ExternalInput')
    out = nc.dram_tensor('out', [B, NLOC], BF, kind='ExternalOutput')

    NU = B * 4  # 8 attention units

    with tile.TileContext(nc) as tc:
        with (
            tc.tile_pool(name='const', bufs=1) as constp,
            tc.tile_pool(name='fs', bufs=1) as fsp,
            tc.tile_pool(name='gat', bufs=1) as gatp,
            tc.tile_pool(name='qk', bufs=1) as qkp,
            tc.tile_pool(name='rel', bufs=1) as relp,
            tc.tile_pool(name='v', bufs=1) as vp,
            tc.tile_pool(name='mlp', bufs=1) as mlpp,
            tc.tile_pool(name='small', bufs=1) as smallp,
            tc.tile_pool(name='ps', bufs=1, space='PSUM') as psp,
            tc.tile_pool(name='psx', bufs=1, space='PSUM') as psxp,
            tc.tile_pool(name='dram', bufs=1, space='DRAM') as dramp,
        ):
            # ---- AllGather feat row shards and the weight blob ----
            featfull = [dramp.tile([NROW, C], BF, name=f'featfull{_b}')
                        for _b in range(B)]
            for _b in range(B):
                ccf_in = dramp.tile([128, 256], BF, name=f'ccf_in{_b}')
                nc.sync.dma_start(out=ccf_in[:], in_=feati[_b, :, :])
                nc.gpsimd.collective_compute(
                    'AllGather', mybir.AluOpType.bypass,
                    replica_groups=[list(range(NCORES))],
                    ins=[ccf_in.opt()], outs=[featfull[_b].opt()],
                )
            wfull = dramp.tile([WBLOB], BF, name='wfull')
            wcc_in = dramp.tile([128, WSH // 128], BF, name='wcc_in')
            nc.sync.dma_start(out=wcc_in[:], in_=wblob[:, :])
            nc.gpsimd.collective_compute(
                'AllGather', mybir.AluOpType.bypass,
                replica_groups=[list(range(NCORES))],
                ins=[wcc_in.opt()], outs=[wfull.opt()],
            )

            # ---- 66x66 edge-replicated 2x2-patch table, built on device ----
            # ptable[b][jy*66+jx] = [c00|c01|c10|c11],
            # c(dy,dx) = feat[b, :, clip(jy-1+dy,0,63), clip(jx-1+dx,0,63)]
            NTAB = 66 * 66
            ptable = [dramp.tile([NTAB, 256], BF, name=f'ptable{_b}')
                      for _b in range(B)]
            for _b in range(B):
                pt_t = ptable[_b][:, :].tensor
                ff_t = featfull[_b][:, :].tensor
                for dy in (0, 1):
                    yr = ([(0, 1, 0), (1, 64, 0), (65, 1, 63)] if dy == 0
                          else [(0, 64, 0), (64, 2, 63)])
                    for dx in (0, 1):
                        xr = ([(0, 1, 0), (1, 64, 0), (65, 1, 63)] if dx == 0
                              else [(0, 64, 0), (64, 2, 63)])
                        qoff = (dy * 2 + dx) * 64
                        for ri, (jy0, ny, sy0) in enumerate(yr):
                            for (jx0, nx, sx0) in xr:
                                dst = bass.AP(
                                    pt_t, (jy0 * 66 + jx0) * 256 + qoff,
                                    [(66 * 256, ny), (256, nx), (1, 64)])
                                src = bass.AP(
                                    ff_t, (sy0 * 64 + sx0) * 64,
                                    [(4096 if ny > 1 and sy0 == 0 else 0, ny),
                                     (64 if nx > 1 and sx0 == 0 else 0, nx),
                                     (1, 64)])
                                # split the build across two DMA queues
                                eng = nc.sync if (dy * 2 + dx + ri) % 2 else nc.gpsimd
                                eng.dma_start(out=dst, in_=src)

            # ---- constant weights to SBUF (from the gathered blob) ----
            # Per corner-pair slot p: Wq.T/QK at rows 2t:2t+2, cols tt*64, so
            # one matmul with the full [8, .] rel tile as lhsT yields
            # [q_{2p} | q_{2p+1}]; wk is block-diagonal for the same pairing.
            wq_sb = constp.tile([8, 2, 128], BF)
            bq_sb = constp.tile([1, 128], BF)
            wk_sb = constp.tile([128, 128], BF)
            bk_sb = constp.tile([1, 128], BF)
            wv_sb = constp.tile([128, 64], BF)   # Wv.T duplicated in both halves
            bv_sb = constp.tile([128, 1], F32)
            w00o_sb = constp.tile([64, 4 * 256], BF)
            w00fp_sb = constp.tile([128, 2, 256], BF)
            w1_sb = constp.tile([128, 2, 256], BF)
            b1_sb = constp.tile([128, B, 2], F32)
            w2_sb = constp.tile([128, 2], BF)
            nc.vector.memset(wq_sb[:], 0.0)
            nc.vector.memset(wk_sb[:], 0.0)
            for _p in range(2):
                for _tt in range(2):
                    _t = 2 * _p + _tt
                    nc.sync.dma_start(
                        out=wq_sb[2 * _t:2 * _t + 2, _p, _tt * 64:(_tt + 1) * 64],
                        in_=wfull[WOFF_WQ:WOFF_BQ])
                nc.sync.dma_start(out=bq_sb[:, _p * 64:(_p + 1) * 64],
                                  in_=wfull[WOFF_BQ:WOFF_WK])
                nc.sync.dma_start(
                    out=wk_sb[_p * 64:(_p + 1) * 64, _p * 64:(_p + 1) * 64],
                    in_=wfull[WOFF_WK:WOFF_BK])
                nc.sync.dma_start(out=bk_sb[:, _p * 64:(_p + 1) * 64],
                                  in_=wfull[WOFF_BK:WOFF_WV])
            nc.sync.dma_start(out=wv_sb[0:64, :], in_=wfull[WOFF_WV:WOFF_W00O])
            nc.sync.dma_start(out=wv_sb[64:128, :], in_=wfull[WOFF_WV:WOFF_W00O])
            nc.sync.dma_start(out=bv_sb[:], in_=bv[:, :])
            for t in range(4):
                nc.sync.dma_start(
                    out=w00o_sb[:, t * 256:(t + 1) * 256],
                    in_=wfull[WOFF_W00O + t * 16384:WOFF_W00O + (t + 1) * 16384])
            for kk in range(2):
                nc.sync.dma_start(
                    out=w00fp_sb[:, kk, :],
                    in_=wfull[WOFF_W00F + kk * 32768:WOFF_W00F + (kk + 1) * 32768])
                nc.sync.dma_start(
                    out=w1_sb[:, kk, :],
                    in_=wfull[WOFF_W1 + kk * 32768:WOFF_W1 + (kk + 1) * 32768])
                for _b in range(B):
                    nc.sync.dma_start(out=b1_sb[:, _b, kk:kk + 1],
                                      in_=b1[_b, kk, :, :])
                nc.sync.dma_start(
                    out=w2_sb[:, kk:kk + 1],
                    in_=wfull[WOFF_W2 + kk * 128:WOFF_W2 + (kk + 1) * 128])

            Sp_sb = constp.tile([64, NU * 64], F32)   # partial logits, all units

            # =========== phases 1+2 per batch: gather, fs, q/k, S ===========
            from concourse.masks import make_identity
            ident_sb = constp.tile([128, 128], BF)
            make_identity(nc, ident_sb[:])


            ones_nl = constp.tile([1, NLOC], BF)
            nc.vector.memset(ones_nl[:], 1.0)

            # Per batch: quarters of 16 pixel-tiles stream through gather ->
            # RBF multiply -> [128,128] pair transposes -> fs pair chunks,
            # and each quarter's q/k matmuls + logit accumulation run right
            # behind it so the PE overlaps the gather instead of idling.
            fsp_all = [[fsp.tile([128, NLOC], BF, name=f'fsp{_b}_{_p}')
                        for _p in range(2)] for _b in range(B)]
            for b in range(B):
                idx16_sb = gatp.tile([128, 64], I16)
                wsm_sb = gatp.tile([128, 4 * 64], BF)
                idx_sb = gatp.tile([128, 64], dt.int32)
                nc.sync.dma_start(out=idx16_sb[:], in_=idx[b, :, :])
                nc.sync.dma_start(out=wsm_sb[:], in_=wsm[b, :, :])
                nc.vector.tensor_copy(out=idx_sb[:], in_=idx16_sb[:])
                rel8_sb = relp.tile([8, NLOC], dt.int8, name='rel8')
                nc.sync.dma_start(out=rel8_sb[:], in_=relq[b, :, :])
                rel_sb = relp.tile([8, NLOC], BF)
                nc.vector.tensor_copy(out=rel_sb[:], in_=rel8_sb[:])

                s_all = psp.tile([64, 4 * 64], F32, name='s_all')
                for q in range(4):
                    g_pm = gatp.tile([128, 16, 4 * C], BF, name=f'g_pm{q % 2}')
                    for o in range(16):
                        nc.gpsimd.indirect_dma_start(
                            out=g_pm[:, o, :], out_offset=None,
                            in_=ptable[b][:, :],
                            in_offset=bass.IndirectOffsetOnAxis(
                                ap=idx_sb[:, q * 16 + o:q * 16 + o + 1], axis=0))
                    wap = wsm_sb[:, q * 64:(q + 1) * 64]
                    wbc = bass.AP(wap.tensor, wap.offset, wap.ap + [(0, C)])
                    nc.vector.tensor_tensor(out=g_pm[:, :, :],
                                            in0=g_pm[:, :, :], in1=wbc,
                                            op=ALU.mult)
                    for p in range(2):
                        for jg in range(4):
                            tp_ps = psp.tile([128, 512], BF, name='tp')
                            for jj in range(4):
                                jl = jg * 4 + jj
                                nc.tensor.transpose(
                                    out=tp_ps[:, jj * 128:(jj + 1) * 128],
                                    in_=g_pm[:, jl, p * 128:(p + 1) * 128],
                                    identity=ident_sb[:])
                            goff = (q * 16 + jg * 4) * 128
                            # alternate engines so neither serializes the chain
                            if (p * 4 + jg) % 2:
                                nc.vector.tensor_copy(
                                    out=fsp_all[b][p][:, goff:goff + 512],
                                    in_=tp_ps[:])
                            else:
                                nc.scalar.copy(
                                    out=fsp_all[b][p][:, goff:goff + 512],
                                    in_=tp_ps[:])
                    # q/k + logit accumulation over this quarter's pixels;
                    # one matmul per pixel-tile covers both corners of a pair
                    for p in range(2):
                        fpt = fsp_all[b][p]
                        for jg in range(4):      # 4 groups of 4 pixel-tiles
                            q_ps = psp.tile([128, 512], F32)
                            k_ps = psp.tile([128, 512], F32)
                            for jj in range(4):
                                j = q * 16 + jg * 4 + jj
                                csl = slice(j * 128, (j + 1) * 128)
                                osl = slice(jj * 128, (jj + 1) * 128)
                                nc.tensor.matmul(
                                    out=q_ps[:, osl],
                                    lhsT=rel_sb[:, csl],
                                    rhs=wq_sb[:, p, :],
                                    start=True, stop=not qk_bias)
                                nc.tensor.matmul(
                                    out=k_ps[:, osl], lhsT=fpt[:, csl],
                                    rhs=wk_sb[:, :],
                                    start=True, stop=not qk_bias)
                                if qk_bias:
                                    nc.tensor.matmul(
                                        out=q_ps[:, osl], lhsT=ones_nl[:, csl],
                                        rhs=bq_sb[:], start=False, stop=True)
                                    nc.tensor.matmul(
                                        out=k_ps[:, osl], lhsT=ones_nl[:, csl],
                                        rhs=bk_sb[:], start=False, stop=True)
                            qs_sb = qkp.tile([128, 512], BF, name=f'qs{jg % 2}')
                            ks_sb = qkp.tile([128, 512], BF, name=f'ks{jg % 2}')
                            nc.scalar.activation(out=qs_sb[:], in_=q_ps[:], func=AF.Relu)
                            nc.vector.tensor_scalar_max(out=ks_sb[:], in0=k_ps[:], scalar1=0.0)
                            for jj in range(4):
                                for tt in range(2):
                                    t = 2 * p + tt
                                    ssl = slice(jj * 128 + tt * 64,
                                                jj * 128 + (tt + 1) * 64)
                                    nc.tensor.matmul(
                                        out=s_all[:, t * 64:(t + 1) * 64],
                                        lhsT=qs_sb[:, ssl],
                                        rhs=ks_sb[:, ssl],
                                        start=(q == 0 and jg == 0 and jj == 0),
                                        stop=(q == 3 and jg == 3 and jj == 3))
                nc.vector.tensor_copy(out=Sp_sb[:, b * 256:(b + 1) * 256],
                                      in_=s_all[:])

            # =========== phase 3: AllReduce of logits ===========
            cc_in = dramp.tile([64, NU * 64], F32)
            cc_out = dramp.tile([64, NU * 64], F32)
            nc.gpsimd.dma_start(out=cc_in[:], in_=Sp_sb[:])
            nc.gpsimd.collective_compute(
                'AllReduce', mybir.AluOpType.add,
                replica_groups=[list(range(NCORES))],
                ins=[cc_in.opt()], outs=[cc_out.opt()],
            )
            S_sb = constp.tile([64, NU * 64], F32)
            nc.gpsimd.dma_start(out=S_sb[:], in_=cc_out[:])

            # =========== phase 4: softmax + A_t^T ===========
            attn_sb = constp.tile([64, NU * 64], BF)
            AT_tiles = []
            for u in range(NU):
                usl = slice(u * 64, (u + 1) * 64)
                mx = smallp.tile([64, 1], F32)
                nmx = smallp.tile([64, 1], F32)
                ex = smallp.tile([64, 64], F32)
                sm = smallp.tile([64, 1], F32)
                rs = smallp.tile([64, 1], F32)
                nc.vector.tensor_reduce(out=mx[:], in_=S_sb[:, usl],
                                        axis=mybir.AxisListType.X, op=ALU.max)
                nc.vector.tensor_scalar_mul(out=nmx[:], in0=mx[:], scalar1=-1.0)
                nc.scalar.activation(out=ex[:], in_=S_sb[:, usl], func=AF.Exp,
                                     bias=nmx[:, 0:1])
                nc.vector.tensor_reduce(out=sm[:], in_=ex[:],
                                        axis=mybir.AxisListType.X, op=ALU.add)
                nc.vector.reciprocal(out=rs[:], in_=sm[:])
                nc.vector.tensor_scalar_mul(out=attn_sb[:, usl], in0=ex[:],
                                            scalar1=rs[:, 0:1])
            # v = relu(Wv fs + bv) is independent of the attention logits:
            # queue the first super-chunk's v for both batches ahead of the
            # AT matmuls so the PE isn't stalled behind the AllReduce, then
            # prefetch v(pc+1) ahead of each MLP chunk.
            def compute_v(b, pc):
                tiles = []
                for p in range(2):
                    vt = vp.tile([128, PCH], BF, name=f'vt{b}_{p}_{pc % 2}')
                    for cc in range(PCH // CHUNK):
                        vsl_l = slice(cc * CHUNK, (cc + 1) * CHUNK)
                        vsl_g = slice(pc * PCH + cc * CHUNK,
                                      pc * PCH + (cc + 1) * CHUNK)
                        v_ps = psp.tile([128, CHUNK], F32)
                        nc.tensor.matmul(out=v_ps[0:64, :], lhsT=wv_sb[0:64, :],
                                         rhs=fsp_all[b][p][0:64, vsl_g],
                                         start=True, stop=True)
                        nc.tensor.matmul(out=v_ps[64:128, :], lhsT=wv_sb[64:128, :],
                                         rhs=fsp_all[b][p][64:128, vsl_g],
                                         start=True, stop=True)
                        nc.scalar.activation(out=vt[:, vsl_l], in_=v_ps[:],
                                             func=AF.Relu, bias=bv_sb[:, 0:1])
                    tiles.append(vt)
                return tiles

            v_next = {_b: compute_v(_b, 0) for _b in range(B)}
            v_next2 = {_b: compute_v(_b, 1) for _b in range(B)}

            for b in range(B):
                for p in range(2):
                    a_full = psp.tile([128, 512], F32, name='misc_ps')
                    for h in range(2):
                        t = 2 * p + h
                        u = b * 4 + t
                        nc.tensor.matmul(
                            out=a_full[h * 64:(h + 1) * 64, 0:256],
                            lhsT=attn_sb[:, u * 64:(u + 1) * 64],
                            rhs=w00o_sb[:, t * 256:(t + 1) * 256],
                            start=True, stop=True)
                    at = constp.tile([128, 256], BF, name=f'atp{b}_{p}')
                    nc.vector.tensor_copy(out=at[:], in_=a_full[:, 0:256])
                    AT_tiles.append(at)

            # =========== phase 5: MLP over resident fs pairs ===========
            for b in range(B):
                bil_sb = smallp.tile([1, NLOC], BF, name='bil_sb')
                nc.sync.dma_start(out=bil_sb[:], in_=bil[b, :][None, :])
                o_row = smallp.tile([1, NLOC], BF, name='o_row')

                for pc in range(NLOC // PCH):
                    v_tiles = v_next[b]
                    v_next[b] = v_next2[b]
                    if pc + 2 < NLOC // PCH:
                        v_next2[b] = compute_v(b, pc + 2)

                    x1_t = [mlpp.tile([128, PCH], BF, name=f'x1_{_m}_{pc % 2}')
                            for _m in range(2)]
                    x2_t = [mlpp.tile([128, PCH], BF, name=f'x2_{_m}_{pc % 2}')
                            for _m in range(2)]
                    for cc in range(PCH // CHUNK):
                        lsl = slice(cc * CHUNK, (cc + 1) * CHUNK)
                        gsl = slice(pc * PCH + cc * CHUNK, pc * PCH + (cc + 1) * CHUNK)
                        for m in range(2):
                            msl = slice(m * 128, (m + 1) * 128)
                            x_ps = psxp.tile([128, CHUNK], F32)
                            for p in range(2):
                                nc.tensor.matmul(
                                    out=x_ps[:],
                                    lhsT=w00fp_sb[:, p, msl],
                                    rhs=fsp_all[b][p][:, gsl],
                                    start=(p == 0), stop=False)
                            for p in range(2):
                                at = AT_tiles[b * 2 + p]
                                nc.tensor.matmul(
                                    out=x_ps[:],
                                    lhsT=at[:, msl],
                                    rhs=v_tiles[p][:, lsl],
                                    start=False, stop=(p == 1))
                            nc.vector.tensor_copy(out=x1_t[m][:, lsl], in_=x_ps[:])
                        # W1 + gelu
                        for m in range(2):
                            msl = slice(m * 128, (m + 1) * 128)
                            x2_ps = psxp.tile([128, CHUNK], F32)
                            for kk in range(2):
                                nc.tensor.matmul(out=x2_ps[:],
                                                 lhsT=w1_sb[:, kk, msl],
                                                 rhs=x1_t[kk][:, lsl],
                                                 start=(kk == 0), stop=(kk == 1))
                            nc.scalar.activation(out=x2_t[m][:, lsl], in_=x2_ps[:],
                                                 func=AF.Gelu, bias=b1_sb[:, b, m:m + 1])
                        # W2; bil add batched once per batch below
                        o_full = psp.tile([64, 512], F32, name='misc_ps')
                        o_ps = o_full[0:1, :]
                        for kk in range(2):
                            nc.tensor.matmul(out=o_ps, lhsT=w2_sb[:, kk:kk + 1],
                                             rhs=x2_t[kk][:, lsl],
                                             start=(kk == 0), stop=(kk == 1))
                        nc.vector.tensor_copy(out=o_row[:, gsl], in_=o_ps)
                nc.vector.tensor_tensor(out=o_row[:], in0=o_row[:],
                                        in1=bil_sb[:], op=ALU.add)
                nc.sync.dma_start(out=out[b, :][None, :], in_=o_row[:])

    nc.compile()
    return nc


# --------------------------------------------------------------------------

def kernel(**inputs) -> np.ndarray:
    from concourse.bass_utils import run_bass_kernel_spmd
    in_maps = _host_prep(inputs)
    qk_bias = bool(np.any(np.asarray(inputs['bq']))
                   or np.any(np.asarray(inputs['bk'])))
    nc = _build(qk_bias)
    res = run_bass_kernel_spmd(nc, in_maps, core_ids=list(range(NCORES)))
    full = np.empty((B, 1, HQ, WQ), np.float32)
    flat = full.reshape(B, NPB)
    for cidx in range(NCORES):
        flat[:, cidx * NLOC:(cidx + 1) * NLOC] = \
            res.results[cidx]['out'].astype(np.float32)
    return full
